# revision 25
# baseline (speedup 1.0000x reference)
"""MoE feed-forward (top-2 routing, 8 experts) on 8 Trainium2 NeuronCores.

Expert-parallel sharding: host computes the (tiny) router + argsort
permutation exactly as the reference does, gathers each expert's token
chunk (pre-transposed to [D, C]), and sends chunk e + expert e's
weights to core e. Each core runs a dense FFN: y = gelu(x @ w1) @ w2,
scaled by the per-row gate weight. Host then inverts the permutation
and sums the top-2 contributions.

Device kernel (per core, SPMD):
  - mm1 in bf16: hT[f,c] = w1[d,f]^T xT, accumulated over 8 d-tiles.
  - gelu on ScalarE, output bf16.
  - mm2 in bf16 (w2 resident in SBUF): y[c,d] += hT[f,c]^T @ w2[f,d],
    accumulated over all 32 f-tiles in PSUM.
  - gate scale applied during the PSUM->SBUF drain on ScalarE.

The kernel is PE-bound: 2048 matmuls x 512 rows = 1.05M PE cycles is
the bf16 floor (~437us at the full 2.4 GHz PE clock; ~530us when the
chip's power management drops the PE to ~2.0 GHz duty). Startup is
bounded by the 8-core-contended HBM staging of the first c-block pair
(~2.5 MB/core). The final drain is shortened by splitting the last
PSUM group's 32-matmul accumulation into two 16-matmul halves: half A
drains (with gate scale) behind half B's matmuls, and DVE
scalar_tensor_tensor ops merge half B (scaled, read straight from
PSUM) into the drained half A quarter-by-quarter, each quarter's
output DMA launching on its own DGE ring while the next quarter
merges. Output DMAs alternate between the two hardware DGE rings.

Startup begins with one tiny (8 KB) DMA on each hardware DGE ring:
they absorb the ~1.5 us DGE/SDMA wake-up latency, and the slightly
later first matmul that results is a net win — the PE then ramps
through the HAM cold-clock window stall-free (measured pre-steady
excess drops from ~6 us to ~4 us, the pure ramp surcharge).
"""

import os
import sys

if "jax" not in sys.modules:
    _jp = os.environ.get("JAX_PLATFORMS", "")
    if _jp and "cpu" not in _jp.split(","):
        os.environ["JAX_PLATFORMS"] = _jp + ",cpu"

import numpy as np

B, T, D, FF, E, TOP_K = 4, 2048, 1024, 4096, 8, 2
N = B * T
NE = N * TOP_K
C_PER = NE // E
P = 128
FT = FF // P
DT = D // P
NCB = 4
CB = C_PER // NCB
NCT = C_PER // P

_cache = {}


MM1_BF16 = True


def _build_program(act_name="Gelu"):
    import concourse.mybir as mybir
    import concourse.tile as tile
    from concourse import bacc

    f32 = mybir.dt.float32
    f32r = mybir.dt.float32r
    bf16 = mybir.dt.bfloat16
    mm1dt = bf16 if MM1_BF16 else f32r
    Act = mybir.ActivationFunctionType
    Alu = mybir.AluOpType

    nc = bacc.Bacc("TRN2", num_devices=E)
    xt_d = nc.dram_tensor("xt", [D, C_PER], mm1dt, kind="ExternalInput")
    w1_d = nc.dram_tensor("w1r", [FT, P, DT, P], mm1dt, kind="ExternalInput")
    w2_d = nc.dram_tensor("w2b", [FF, D], bf16, kind="ExternalInput")
    sw_d = nc.dram_tensor("swt", [P, NCT], f32, kind="ExternalInput")
    # y leaves the device in bf16: the 2e-2 tolerance has ~5x headroom
    # over the ~2e-3 this quantization adds, and halving the output
    # stream (8->4 MB/core) shortens both the drain DMAs during mm2
    # and the trailing output-DMA completion that ends the measured
    # execution window
    y_d = nc.dram_tensor("y", [C_PER, D], bf16, kind="ExternalOutput")

    with tile.TileContext(nc) as tc:
        with tc.tile_pool(name="sb", bufs=1) as sb, \
             tc.tile_pool(name="w1p", bufs=8) as w1p, \
             tc.tile_pool(name="yop", bufs=3) as yop, \
             tc.tile_pool(name="psh", bufs=4, space="PSUM") as psh, \
             tc.tile_pool(name="psy", bufs=4, space="PSUM") as psy:
            cpool = xtp = w2p = hp = sb

            w2t = [None] * FT
            ht = [None] * FT
            xt = {}

            def load_xt(d, pair, eng=None):
                t = xtp.tile([P, 2 * CB], mm1dt, tag=f"xt_{d}_{pair}",
                             name=f"xt_{d}_{pair}")
                (eng or nc.sync).dma_start(
                    out=t,
                    in_=xt_d.ap()[d * P:(d + 1) * P,
                                  pair * 2 * CB:(pair + 1) * 2 * CB])
                xt[(d, pair)] = t

            # tiny pipe-warmer DMAs: absorb the ~1.5us DGE/SDMA
            # wake-up latency on both hardware rings so the first real
            # transfers start flowing immediately behind them
            swt = cpool.tile([P, NCT], f32, tag="swt")
            nc.sync.dma_start(out=swt, in_=sw_d.ap())
            swt_w = cpool.tile([P, NCT], f32, tag="swt_w", name="swt_w")
            nc.scalar.dma_start(out=swt_w, in_=sw_d.ap())
            w1t0a = w1p.tile([P, DT // 2, P], mm1dt, tag="w1a", name="w1a")
            nc.sync.dma_start(out=w1t0a, in_=w1_d.ap()[0, :, 0:DT // 2])
            load_xt(0, 0, eng=nc.scalar)
            w1t0b = w1p.tile([P, DT // 2, P], mm1dt, tag="w1b", name="w1b")
            nc.scalar.dma_start(out=w1t0b, in_=w1_d.ap()[0, :, DT // 2:DT])
            first_eng = [None, nc.sync, nc.scalar, nc.sync,
                         nc.scalar, nc.sync, nc.scalar, nc.sync]
            for d in range(1, DT):
                load_xt(d, 0, eng=first_eng[d])
            def load_w2(ft):
                t = w2p.tile([P, D], bf16, tag=f"w2_{ft}",
                             name=f"w2_{ft}")
                nc.sync.dma_start(
                    out=t, in_=w2_d.ap()[ft * P:(ft + 1) * P, :])
                w2t[ft] = t

            ht = {}
            ndrain = 0
            for pair in range(NCB // 2):
                cbs = (2 * pair, 2 * pair + 1)
                for ft in range(FT):
                    if pair == 0 and ft == 0:
                        def w1ap(d, a=w1t0a, b=w1t0b):
                            t = a if d < DT // 2 else b
                            return t[:, d % (DT // 2), :]
                    else:
                        w1t = w1p.tile([P, DT, P], mm1dt, tag="w1",
                                       name="w1t")
                        nc.sync.dma_start(out=w1t, in_=w1_d.ap()[ft])

                        def w1ap(d, t=w1t):
                            return t[:, d, :]
                    if pair == 0:
                        load_w2(ft)
                    if pair == 0 and ft == 0:
                        # first f-tile: run both c-halves' d0-3 chains
                        # before any d4-7 chain. The d0-3 xT tiles and
                        # w1 half arrive ~1 MB sooner than the full
                        # working set, so a startup-contended core gets
                        # ~3.4 us of schedulable work while d4-7 and
                        # the second w1 half are still in flight.
                        # Accumulation state lives per PSUM bank, so
                        # the two open groups interleave legally.
                        hps2 = [psh.tile([P, CB], f32, tag="psh",
                                         name=f"hps0_{ch}")
                                for ch in range(2)]
                        for dlo, dhi in ((0, DT // 2), (DT // 2, DT)):
                            for ch in range(2):
                                for d in range(dlo, dhi):
                                    nc.tensor.matmul(
                                        hps2[ch], w1ap(d),
                                        xt[(d, pair)][:,
                                                      ch * CB:(ch + 1) * CB],
                                        start=(d == 0),
                                        stop=(d == DT - 1))
                        for ch in range(2):
                            h_t = hp.tile([P, CB], bf16,
                                          tag=f"h_{ch}_{ft}",
                                          name=f"h_{ch}_{ft}")
                            nc.scalar.activation(h_t, hps2[ch],
                                                 getattr(Act, act_name))
                            ht[(ch, ft)] = h_t
                        continue
                    for ch, cb in enumerate(cbs):
                        hps = psh.tile([P, CB], f32, tag="psh", name="hps")
                        for d in range(DT):
                            nc.tensor.matmul(
                                hps, w1ap(d),
                                xt[(d, pair)][:, ch * CB:(ch + 1) * CB],
                                start=(d == 0), stop=(d == DT - 1))
                        h_t = hp.tile([P, CB], bf16, tag=f"h_{ch}_{ft}",
                                      name=f"h_{ch}_{ft}")
                        nc.scalar.activation(h_t, hps,
                                             getattr(Act, act_name))
                        ht[(ch, ft)] = h_t
                for ch, cb in enumerate(cbs):
                    for db in range(2):
                        d0 = db * (D // 2)
                        for ct in range(CB // P):
                            g = cb * (CB // P) + ct
                            last = (cb == NCB - 1 and db == 1
                                    and ct == CB // P - 1)
                            if last:
                                # split the final accumulation so half
                                # A drains behind half B's matmuls and
                                # only the DVE merge + output DMA stay
                                # exposed after the last matmul
                                ypsA = psy.tile([P, D // 2], f32,
                                                tag="psy", name="ypsA")
                                for ft in range(FT // 2):
                                    nc.tensor.matmul(
                                        ypsA,
                                        ht[(ch, ft)][:, ct * P:(ct + 1) * P],
                                        w2t[ft][:, d0:d0 + D // 2],
                                        start=(ft == 0),
                                        stop=(ft == FT // 2 - 1))
                                yoA = yop.tile([P, D // 2], bf16,
                                               tag="yo", name="yoA")
                                nc.scalar.activation(
                                    yoA, ypsA, Act.Copy,
                                    scale=swt[:, g:g + 1])
                                ypsB = psy.tile([P, D // 2], f32,
                                                tag="psy", name="ypsB")
                                for ft in range(FT // 2, FT):
                                    nc.tensor.matmul(
                                        ypsB,
                                        ht[(ch, ft)][:, ct * P:(ct + 1) * P],
                                        w2t[ft][:, d0:d0 + D // 2],
                                        start=(ft == FT // 2),
                                        stop=(ft == FT - 1))
                                yo = yop.tile([P, D // 2], bf16,
                                              tag="yo", name="yoS")
                                quart = D // 8
                                for q in range(4):
                                    q0 = q * quart
                                    nc.vector.scalar_tensor_tensor(
                                        yo[:, q0:q0 + quart],
                                        ypsB[:, q0:q0 + quart],
                                        swt[:, g:g + 1],
                                        yoA[:, q0:q0 + quart],
                                        Alu.mult, Alu.add)
                                    eng = (nc.sync if q % 2 == 0
                                           else nc.scalar)
                                    eng.dma_start(
                                        out=y_d.ap()[
                                            g * P:(g + 1) * P,
                                            d0 + q0:d0 + q0 + quart],
                                        in_=yo[:, q0:q0 + quart])
                            else:
                                yps = psy.tile([P, D // 2], f32,
                                               tag="psy", name="yps")
                                for ft in range(FT):
                                    nc.tensor.matmul(
                                        yps,
                                        ht[(ch, ft)][:, ct * P:(ct + 1) * P],
                                        w2t[ft][:, d0:d0 + D // 2],
                                        start=(ft == 0),
                                        stop=(ft == FT - 1))
                                yo = yop.tile([P, D // 2], bf16,
                                              tag="yo", name="yo")
                                nc.scalar.activation(yo, yps,
                                                     Act.Copy,
                                                     scale=swt[:, g:g + 1])
                                # alternate output DMAs across the two
                                # hardware DGE rings
                                eng = (nc.sync if ndrain % 2 == 0
                                       else nc.scalar)
                                ndrain += 1
                                eng.dma_start(
                                    out=y_d.ap()[g * P:(g + 1) * P,
                                                 d0:d0 + D // 2],
                                    in_=yo)
                            if (ch == 0 and db == 0 and ct == 0
                                    and pair + 1 < NCB // 2):
                                for d in range(DT):
                                    load_xt(d, pair + 1)
    nc.compile()
    return nc


def _get_program():
    if "nc" not in _cache:
        _cache["nc"] = _build_program()
    return _cache["nc"]


def _routing(xf, router_w):
    """Replicate the reference gating bit-exactly where it matters.

    Returns (rev, sw, sort_idx). The top-k *selection* must match the
    reference exactly (it is discrete); we therefore compute the router
    logits with jax when available, mirroring reference.py. The softmax
    and sort bookkeeping is continuous or exactly replicable in numpy.
    """
    try:
        import jax
        import jax.numpy as jnp

        def _gate():
            logits = jnp.asarray(xf) @ jnp.asarray(router_w).T
            return jax.lax.top_k(logits, TOP_K)

        try:
            cpu = jax.devices("cpu")[0]
            with jax.default_device(cpu):
                tv, ti = _gate()
        except Exception:
            tv, ti = _gate()
        topv = np.asarray(tv, dtype=np.float32)
        topi = np.asarray(ti)
    except Exception:
        logits = xf @ router_w.T
        # top-2 with jax tie-breaking (lower index wins)
        i0 = np.argmax(logits, axis=-1)
        v0 = np.take_along_axis(logits, i0[:, None], axis=-1)[:, 0]
        masked = logits.copy()
        np.put_along_axis(masked, i0[:, None], -np.inf, axis=-1)
        i1 = np.argmax(masked, axis=-1)
        v1 = np.take_along_axis(logits, i1[:, None], axis=-1)[:, 0]
        topi = np.stack([i0, i1], axis=-1)
        topv = np.stack([v0, v1], axis=-1).astype(np.float32)

    # softmax over the two gate logits, float32
    m = topv.max(axis=-1, keepdims=True)
    e = np.exp(topv - m, dtype=np.float32)
    topw = (e / e.sum(axis=-1, keepdims=True)).astype(np.float32)

    idx_flat = topi.reshape(-1)
    w_flat = topw.reshape(-1)
    # stable argsort of integer keys is uniquely determined by the keys
    sort_idx = np.argsort(idx_flat, kind="stable")
    src = np.repeat(np.arange(N), TOP_K)
    rev = src[sort_idx]
    sw = w_flat[sort_idx]
    return rev, sw, sort_idx


def _ensure_axon_hooks():
    """Make `antenv.axon_hooks` importable so run_bass_kernel_spmd's
    trace path degrades gracefully (or works, if the axon boot shim is
    available) instead of crashing on ImportError."""
    try:
        import antenv.axon_hooks  # noqa: F401
        return
    except ImportError:
        pass
    import sys
    import types
    mod = types.ModuleType("antenv.axon_hooks")
    state = {"hook": None}
    mod.set_axon_ntff_profile_hook = lambda h: state.update(hook=h)
    mod.get_axon_ntff_profile_hook = lambda: state["hook"]
    try:
        import antenv
        sys.modules["antenv.axon_hooks"] = mod
        antenv.axon_hooks = mod
    except ImportError:
        return
    try:
        from trn_agent_boot.trn_boot import _ntff_profile_via_ctypes
        h = _ntff_profile_via_ctypes("/opt/axon/libaxon_pjrt.so")
        if h is not None:
            mod.set_axon_ntff_profile_hook(h)
            import concourse.bass_utils as bu
            bu.upload_artifacts = lambda tmpdir: "local://" + str(tmpdir)
    except Exception:
        pass


def kernel(x, router_w, w1, w2):
    import ml_dtypes
    from concourse import bass_utils
    _ensure_axon_hooks()

    xf = np.ascontiguousarray(x.reshape(-1, D), dtype=np.float32)
    rev, sw, sort_idx = _routing(xf, router_w)

    nc = _get_program()

    in_maps = []
    for e in range(E):
        rows = rev[e * C_PER:(e + 1) * C_PER]
        xct = np.ascontiguousarray(xf[rows].T)
        w1r = np.ascontiguousarray(
            w1[e].reshape(DT, P, FT, P).transpose(2, 1, 0, 3))
        if MM1_BF16:
            xct = xct.astype(ml_dtypes.bfloat16)
            w1r = w1r.astype(ml_dtypes.bfloat16)
        w2b = np.ascontiguousarray(w2[e].astype(ml_dtypes.bfloat16))
        swt = np.ascontiguousarray(
            sw[e * C_PER:(e + 1) * C_PER].reshape(NCT, P).T)
        in_maps.append({"xt": xct, "w1r": w1r, "w2b": w2b, "swt": swt})

    r = bass_utils.run_bass_kernel_spmd(nc, in_maps, core_ids=list(range(E)))
    _cache["last_result"] = r

    y_sorted = np.empty((NE, D), dtype=np.float32)
    for e in range(E):
        y_sorted[e * C_PER:(e + 1) * C_PER] = np.asarray(
            r.results[e]["y"], dtype=np.float32)

    y_expanded = np.empty_like(y_sorted)
    y_expanded[sort_idx] = y_sorted
    out = y_expanded.reshape(N, TOP_K, D).sum(axis=1)
    return out.reshape(B, T, D)


# revision 26
# speedup vs baseline: 1.0065x; 1.0065x over previous
"""MoE feed-forward (top-2 routing, 8 experts) on 8 Trainium2 NeuronCores.

Expert-parallel sharding: host computes the (tiny) router + argsort
permutation exactly as the reference does, gathers each expert's token
chunk (pre-transposed to [D, C]), and sends chunk e + expert e's
weights to core e. Each core runs a dense FFN: y = gelu(x @ w1) @ w2,
scaled by the per-row gate weight. Host then inverts the permutation
and sums the top-2 contributions.

Device kernel (per core, SPMD):
  - mm1 in bf16: hT[f,c] = w1[d,f]^T xT, accumulated over 8 d-tiles.
  - gelu on ScalarE, output bf16.
  - mm2 in bf16 (w2 resident in SBUF): y[c,d] += hT[f,c]^T @ w2[f,d],
    accumulated over all 32 f-tiles in PSUM.
  - gate scale applied during the PSUM->SBUF drain on ScalarE.

The kernel is PE-bound: 2048 matmuls x 512 rows = 1.05M PE cycles is
the bf16 floor (~437us at the full 2.4 GHz PE clock; ~530us when the
chip's power management drops the PE to ~2.0 GHz duty). Startup is
bounded by the 8-core-contended HBM staging of the first c-block pair
(~2.5 MB/core). The final drain is shortened by splitting the last
PSUM group's 32-matmul accumulation into two 16-matmul halves: half A
drains (with gate scale) behind half B's matmuls, and DVE
scalar_tensor_tensor ops merge half B (scaled, read straight from
PSUM) into the drained half A quarter-by-quarter, each quarter's
output DMA launching on its own DGE ring while the next quarter
merges. Output DMAs alternate between the two hardware DGE rings.

Startup begins with one tiny (8 KB) DMA on each hardware DGE ring:
they absorb the ~1.5 us DGE/SDMA wake-up latency, and the slightly
later first matmul that results is a net win — the PE then ramps
through the HAM cold-clock window stall-free (measured pre-steady
excess drops from ~6 us to ~4 us, the pure ramp surcharge).
"""

import os
import sys

if "jax" not in sys.modules:
    _jp = os.environ.get("JAX_PLATFORMS", "")
    if _jp and "cpu" not in _jp.split(","):
        os.environ["JAX_PLATFORMS"] = _jp + ",cpu"

import numpy as np

B, T, D, FF, E, TOP_K = 4, 2048, 1024, 4096, 8, 2
N = B * T
NE = N * TOP_K
C_PER = NE // E
P = 128
FT = FF // P
DT = D // P
NCB = 4
CB = C_PER // NCB
NCT = C_PER // P

_cache = {}


MM1_BF16 = True


def _build_program(act_name="Gelu"):
    import concourse.mybir as mybir
    import concourse.tile as tile
    from concourse import bacc

    f32 = mybir.dt.float32
    f32r = mybir.dt.float32r
    bf16 = mybir.dt.bfloat16
    mm1dt = bf16 if MM1_BF16 else f32r
    Act = mybir.ActivationFunctionType
    Alu = mybir.AluOpType

    nc = bacc.Bacc("TRN2", num_devices=E)
    xt_d = nc.dram_tensor("xt", [D, C_PER], mm1dt, kind="ExternalInput")
    w1_d = nc.dram_tensor("w1r", [FT, P, DT, P], mm1dt, kind="ExternalInput")
    w2_d = nc.dram_tensor("w2b", [FF, D], bf16, kind="ExternalInput")
    sw_d = nc.dram_tensor("swt", [P, NCT], f32, kind="ExternalInput")
    # y leaves the device in bf16: the 2e-2 tolerance has ~5x headroom
    # over the ~2e-3 this quantization adds, and halving the output
    # stream (8->4 MB/core) shortens both the drain DMAs during mm2
    # and the trailing output-DMA completion that ends the measured
    # execution window
    y_d = nc.dram_tensor("y", [C_PER, D], bf16, kind="ExternalOutput")

    with tile.TileContext(nc) as tc:
        with tc.tile_pool(name="sb", bufs=1) as sb, \
             tc.tile_pool(name="w1p", bufs=8) as w1p, \
             tc.tile_pool(name="yop", bufs=3) as yop, \
             tc.tile_pool(name="psh", bufs=4, space="PSUM") as psh, \
             tc.tile_pool(name="psy", bufs=4, space="PSUM") as psy:
            cpool = xtp = w2p = hp = sb

            w2t = [None] * FT
            ht = [None] * FT
            xt = {}

            def load_xt(d, pair, eng=None):
                t = xtp.tile([P, 2 * CB], mm1dt, tag=f"xt_{d}_{pair}",
                             name=f"xt_{d}_{pair}")
                (eng or nc.sync).dma_start(
                    out=t,
                    in_=xt_d.ap()[d * P:(d + 1) * P,
                                  pair * 2 * CB:(pair + 1) * 2 * CB])
                xt[(d, pair)] = t

            # tiny pipe-warmer DMAs: absorb the ~1.5us DGE/SDMA
            # wake-up latency on both hardware rings so the first real
            # transfers start flowing immediately behind them
            swt = cpool.tile([P, NCT], f32, tag="swt")
            nc.sync.dma_start(out=swt, in_=sw_d.ap())
            swt_w = cpool.tile([P, NCT], f32, tag="swt_w", name="swt_w")
            nc.scalar.dma_start(out=swt_w, in_=sw_d.ap())
            w1t0a = w1p.tile([P, DT // 2, P], mm1dt, tag="w1a", name="w1a")
            nc.sync.dma_start(out=w1t0a, in_=w1_d.ap()[0, :, 0:DT // 2])
            load_xt(0, 0, eng=nc.scalar)
            w1t0b = w1p.tile([P, DT // 2, P], mm1dt, tag="w1b", name="w1b")
            nc.scalar.dma_start(out=w1t0b, in_=w1_d.ap()[0, :, DT // 2:DT])
            first_eng = [None, nc.sync, nc.scalar, nc.sync,
                         nc.scalar, nc.sync, nc.scalar, nc.sync]
            for d in range(1, DT):
                load_xt(d, 0, eng=first_eng[d])
            def load_w2(ft):
                t = w2p.tile([P, D], bf16, tag=f"w2_{ft}",
                             name=f"w2_{ft}")
                nc.sync.dma_start(
                    out=t, in_=w2_d.ap()[ft * P:(ft + 1) * P, :])
                w2t[ft] = t

            ht = {}
            ndrain = 0
            for pair in range(NCB // 2):
                cbs = (2 * pair, 2 * pair + 1)
                for ft in range(FT):
                    if pair == 0 and ft == 0:
                        def w1ap(d, a=w1t0a, b=w1t0b):
                            t = a if d < DT // 2 else b
                            return t[:, d % (DT // 2), :]
                    else:
                        w1t = w1p.tile([P, DT, P], mm1dt, tag="w1",
                                       name="w1t")
                        nc.sync.dma_start(out=w1t, in_=w1_d.ap()[ft])

                        def w1ap(d, t=w1t):
                            return t[:, d, :]
                    if pair == 0:
                        load_w2(ft)
                    for ch, cb in enumerate(cbs):
                        hps = psh.tile([P, CB], f32, tag="psh", name="hps")
                        for d in range(DT):
                            nc.tensor.matmul(
                                hps, w1ap(d),
                                xt[(d, pair)][:, ch * CB:(ch + 1) * CB],
                                start=(d == 0), stop=(d == DT - 1))
                        h_t = hp.tile([P, CB], bf16, tag=f"h_{ch}_{ft}",
                                      name=f"h_{ch}_{ft}")
                        nc.scalar.activation(h_t, hps,
                                             getattr(Act, act_name))
                        ht[(ch, ft)] = h_t
                for ch, cb in enumerate(cbs):
                    for db in range(2):
                        d0 = db * (D // 2)
                        for ct in range(CB // P):
                            g = cb * (CB // P) + ct
                            last = (cb == NCB - 1 and db == 1
                                    and ct == CB // P - 1)
                            if last:
                                # split the final accumulation so half
                                # A drains behind half B's matmuls and
                                # only the DVE merge + output DMA stay
                                # exposed after the last matmul
                                ypsA = psy.tile([P, D // 2], f32,
                                                tag="psy", name="ypsA")
                                for ft in range(FT // 2):
                                    nc.tensor.matmul(
                                        ypsA,
                                        ht[(ch, ft)][:, ct * P:(ct + 1) * P],
                                        w2t[ft][:, d0:d0 + D // 2],
                                        start=(ft == 0),
                                        stop=(ft == FT // 2 - 1))
                                yoA = yop.tile([P, D // 2], bf16,
                                               tag="yo", name="yoA")
                                nc.scalar.activation(
                                    yoA, ypsA, Act.Copy,
                                    scale=swt[:, g:g + 1])
                                ypsB = psy.tile([P, D // 2], f32,
                                                tag="psy", name="ypsB")
                                for ft in range(FT // 2, FT):
                                    nc.tensor.matmul(
                                        ypsB,
                                        ht[(ch, ft)][:, ct * P:(ct + 1) * P],
                                        w2t[ft][:, d0:d0 + D // 2],
                                        start=(ft == FT // 2),
                                        stop=(ft == FT - 1))
                                yo = yop.tile([P, D // 2], bf16,
                                              tag="yo", name="yoS")
                                quart = D // 8
                                for q in range(4):
                                    q0 = q * quart
                                    nc.vector.scalar_tensor_tensor(
                                        yo[:, q0:q0 + quart],
                                        ypsB[:, q0:q0 + quart],
                                        swt[:, g:g + 1],
                                        yoA[:, q0:q0 + quart],
                                        Alu.mult, Alu.add)
                                    eng = (nc.sync if q % 2 == 0
                                           else nc.scalar)
                                    eng.dma_start(
                                        out=y_d.ap()[
                                            g * P:(g + 1) * P,
                                            d0 + q0:d0 + q0 + quart],
                                        in_=yo[:, q0:q0 + quart])
                            else:
                                yps = psy.tile([P, D // 2], f32,
                                               tag="psy", name="yps")
                                for ft in range(FT):
                                    nc.tensor.matmul(
                                        yps,
                                        ht[(ch, ft)][:, ct * P:(ct + 1) * P],
                                        w2t[ft][:, d0:d0 + D // 2],
                                        start=(ft == 0),
                                        stop=(ft == FT - 1))
                                yo = yop.tile([P, D // 2], bf16,
                                              tag="yo", name="yo")
                                nc.scalar.activation(yo, yps,
                                                     Act.Copy,
                                                     scale=swt[:, g:g + 1])
                                # alternate output DMAs across the two
                                # hardware DGE rings
                                eng = (nc.sync if ndrain % 2 == 0
                                       else nc.scalar)
                                ndrain += 1
                                eng.dma_start(
                                    out=y_d.ap()[g * P:(g + 1) * P,
                                                 d0:d0 + D // 2],
                                    in_=yo)
                            if (ch == 0 and db == 0 and ct == 0
                                    and pair + 1 < NCB // 2):
                                for d in range(DT):
                                    load_xt(d, pair + 1)
    nc.compile()
    return nc


def _get_program():
    if "nc" not in _cache:
        _cache["nc"] = _build_program()
    return _cache["nc"]


def _routing(xf, router_w):
    """Replicate the reference gating bit-exactly where it matters.

    Returns (rev, sw, sort_idx). The top-k *selection* must match the
    reference exactly (it is discrete); we therefore compute the router
    logits with jax when available, mirroring reference.py. The softmax
    and sort bookkeeping is continuous or exactly replicable in numpy.
    """
    try:
        import jax
        import jax.numpy as jnp

        def _gate():
            logits = jnp.asarray(xf) @ jnp.asarray(router_w).T
            return jax.lax.top_k(logits, TOP_K)

        try:
            cpu = jax.devices("cpu")[0]
            with jax.default_device(cpu):
                tv, ti = _gate()
        except Exception:
            tv, ti = _gate()
        topv = np.asarray(tv, dtype=np.float32)
        topi = np.asarray(ti)
    except Exception:
        logits = xf @ router_w.T
        # top-2 with jax tie-breaking (lower index wins)
        i0 = np.argmax(logits, axis=-1)
        v0 = np.take_along_axis(logits, i0[:, None], axis=-1)[:, 0]
        masked = logits.copy()
        np.put_along_axis(masked, i0[:, None], -np.inf, axis=-1)
        i1 = np.argmax(masked, axis=-1)
        v1 = np.take_along_axis(logits, i1[:, None], axis=-1)[:, 0]
        topi = np.stack([i0, i1], axis=-1)
        topv = np.stack([v0, v1], axis=-1).astype(np.float32)

    # softmax over the two gate logits, float32
    m = topv.max(axis=-1, keepdims=True)
    e = np.exp(topv - m, dtype=np.float32)
    topw = (e / e.sum(axis=-1, keepdims=True)).astype(np.float32)

    idx_flat = topi.reshape(-1)
    w_flat = topw.reshape(-1)
    # stable argsort of integer keys is uniquely determined by the keys
    sort_idx = np.argsort(idx_flat, kind="stable")
    src = np.repeat(np.arange(N), TOP_K)
    rev = src[sort_idx]
    sw = w_flat[sort_idx]
    return rev, sw, sort_idx


def _ensure_axon_hooks():
    """Make `antenv.axon_hooks` importable so run_bass_kernel_spmd's
    trace path degrades gracefully (or works, if the axon boot shim is
    available) instead of crashing on ImportError."""
    try:
        import antenv.axon_hooks  # noqa: F401
        return
    except ImportError:
        pass
    import sys
    import types
    mod = types.ModuleType("antenv.axon_hooks")
    state = {"hook": None}
    mod.set_axon_ntff_profile_hook = lambda h: state.update(hook=h)
    mod.get_axon_ntff_profile_hook = lambda: state["hook"]
    try:
        import antenv
        sys.modules["antenv.axon_hooks"] = mod
        antenv.axon_hooks = mod
    except ImportError:
        return
    try:
        from trn_agent_boot.trn_boot import _ntff_profile_via_ctypes
        h = _ntff_profile_via_ctypes("/opt/axon/libaxon_pjrt.so")
        if h is not None:
            mod.set_axon_ntff_profile_hook(h)
            import concourse.bass_utils as bu
            bu.upload_artifacts = lambda tmpdir: "local://" + str(tmpdir)
    except Exception:
        pass


def kernel(x, router_w, w1, w2):
    import ml_dtypes
    from concourse import bass_utils
    _ensure_axon_hooks()

    xf = np.ascontiguousarray(x.reshape(-1, D), dtype=np.float32)
    rev, sw, sort_idx = _routing(xf, router_w)

    nc = _get_program()

    in_maps = []
    for e in range(E):
        rows = rev[e * C_PER:(e + 1) * C_PER]
        xct = np.ascontiguousarray(xf[rows].T)
        w1r = np.ascontiguousarray(
            w1[e].reshape(DT, P, FT, P).transpose(2, 1, 0, 3))
        if MM1_BF16:
            xct = xct.astype(ml_dtypes.bfloat16)
            w1r = w1r.astype(ml_dtypes.bfloat16)
        w2b = np.ascontiguousarray(w2[e].astype(ml_dtypes.bfloat16))
        swt = np.ascontiguousarray(
            sw[e * C_PER:(e + 1) * C_PER].reshape(NCT, P).T)
        in_maps.append({"xt": xct, "w1r": w1r, "w2b": w2b, "swt": swt})

    r = bass_utils.run_bass_kernel_spmd(nc, in_maps, core_ids=list(range(E)))
    _cache["last_result"] = r

    y_sorted = np.empty((NE, D), dtype=np.float32)
    for e in range(E):
        y_sorted[e * C_PER:(e + 1) * C_PER] = np.asarray(
            r.results[e]["y"], dtype=np.float32)

    y_expanded = np.empty_like(y_sorted)
    y_expanded[sort_idx] = y_sorted
    out = y_expanded.reshape(N, TOP_K, D).sum(axis=1)
    return out.reshape(B, T, D)
